# revision 2
# baseline (speedup 1.0000x reference)
"""Trainium2 Bass kernel for windowless 3D relative-position attention.

Full-input contract: kernel(**inputs) takes the unsharded numpy inputs and
returns the full [4, 2048, 256] output. Internally shards across 8 NeuronCores
as (batch b = core//2) x (head-group g = core%2, 4 heads each).

v2 design (ACT-exp is the pacemaker; PE assumed cold at 1.2 GHz due to HAM):
  - expBT resident in SBUF ([128, 16, 2048] fp16) loaded once.
  - scores per head-pair tile [128, 2, 512] fp32 (2 PSUM banks), one shared
    tag ring of 2 bufs (4 banks) -> the 4 per-m score matmuls (row bands
    0..3 via tile_position) can fly concurrently while ACT drains m-1.
  - ONE ACT exp per head-pair (FD=1024, PSUM->SBUF fp16), writing halves of
    a [128, 4, 512] aw tile; ONE DVE mul (FD=2048) applies exp(bias) to all
    4 heads per m-tile (eb broadcast via stride-0 AP).
  - AV lhsT is [128 keys, 64]: cols 0:32 = v, 32:64 = ones -> the softmax
    denominator comes out REPLICATED on 32 psum partitions; normalization is
    reciprocal_approx_fast per oa bank + one fp32 mul per head (no DRAM
    round-trips, no 1-partition copies).
  - oa: 2 heads per bank at PE column offsets 0/64 (2-way AV concurrency),
    2 banks per quarter, double-buffered (oa0/oa1 tags x 2 bufs) = 4 banks.
    PSUM total = 4 (sc ring) + 4 (oa) = 8.
  - quarter-q normalize steps (2 recip + 4 mul) interleave into quarter
    q+1's m-stream; out-projection runs at the tail with copies alternating
    scalar/vector engines, output DMA on the gpsimd queue.
  - dummy exp activation issued at t~0 so the ~2.7us ACT table load hides
    under the input DMAs.

The bias add is factored through the exponential: exp(s+bias) =
exp(s)*exp(bias), with exp(bias.T - C_SHIFT) precomputed on host in fp16
(C_SHIFT keeps products in fp16 range; it cancels in the softmax ratio).

Hardware constraints learned (do not re-attempt blindly): matmul PSUM out
is limited to one 2KB bank (512 fp32 cols); engine APs must start on
partition 0/32/64/96 and fit the quadrant; DMA cannot touch PSUM; DMA
partition stride must be 1 (stride-0 broadcast only from DRAM); tile pools
release LIFO; walrus ldw-opt is incompatible with Bass IR; fp8 DoubleRow
gives no column-rate gain at K<128; GPSIMD DGE adds latency in
dependency-critical chains; HAM keeps PE at 1.2 GHz unless continuously
busy >=3.4us (ACT-gated cadence never re-warms it).
"""

import os
import sys
from contextlib import ExitStack

import numpy as np

sys.path.insert(0, "/opt/trn_rl_repo")

import concourse.bass as bass
import concourse.bacc as bacc
import concourse.tile as tile
from concourse import mybir
from concourse.bass_utils import run_bass_kernel_spmd

# Problem constants (hardcoded per contract)
B = 4
N = 2048
INP = 256
OUP = 256
HEADS = 8
DIM_HEAD = 32
SCALE = DIM_HEAD ** -0.5
HL = 4            # heads per core
MT = N // 128     # 16 m-tiles (keys)
NQ = 4            # 512-wide n (query) quarters
NQW = 512
C_SHIFT = 4.0

f32 = mybir.dt.float32
f16 = mybir.dt.float16

_LAST = {"exec_time_ns": None}


def _build_nc():
    nc = bacc.Bacc("TRN2", target_bir_lowering=False, debug=False)
    xT_d = nc.dram_tensor("xT", [2, 128, N], f16, kind="ExternalInput")
    wqk_d = nc.dram_tensor("w_qk", [2, 128, 256], f16, kind="ExternalInput")
    wv_d = nc.dram_tensor("w_v", [2, 128, 128], f16, kind="ExternalInput")
    wout_d = nc.dram_tensor("w_out2", [128, 256], f16, kind="ExternalInput")
    ebt_d = nc.dram_tensor("expbt", [N, N], f16, kind="ExternalInput")
    out_d = nc.dram_tensor("partial", [N, OUP], f32, kind="ExternalOutput")

    with ExitStack() as ctx:
        tc = ctx.enter_context(tile.TileContext(nc))
        consts = ctx.enter_context(tc.tile_pool(name="consts", bufs=1))

        ebt = consts.tile([128, MT, N], f16)          # [m%128, mtile, n]
        xT = consts.tile([128, 2, N], f16)
        wqk = consts.tile([128, 2, 256], f16)
        wv = consts.tile([128, 2, 128], f16)
        woutd = consts.tile([128, 256], f16)
        qkT = consts.tile([128, 2, N], f16)           # [:,0,:]=qT  [:,1,:]=kT
        vsb = consts.tile([128, MT, HL, 64], f16)     # [key, mtile, head, v|ones]
        aoutT = consts.tile([128, N], f16)            # [(h,d), n] normalized
        dummy = consts.tile([128, 8], f32)

        # dummy exp right away: the ~2.7us ACT table load hides under DMAs
        nc.vector.memset(dummy[:], 1.0)
        nc.scalar.activation(
            out=dummy[:, 0:4], in_=dummy[:, 4:8],
            func=mybir.ActivationFunctionType.Exp, scale=1.0,
        )

        # first columns of x + w_qk first (unblocks projection chunk 0),
        # then the rest; the big ebt stream follows behind
        for kk in range(2):
            nc.sync.dma_start(out=xT[:, kk, 0:N // 2], in_=xT_d[kk, :, 0:N // 2])
        for kk in range(2):
            nc.sync.dma_start(out=wqk[:, kk, :], in_=wqk_d[kk])
        for kk in range(2):
            nc.sync.dma_start(out=xT[:, kk, N // 2:N], in_=xT_d[kk, :, N // 2:N])
        for kk in range(2):
            nc.sync.dma_start(out=wv[:, kk, :], in_=wv_d[kk])
        nc.sync.dma_start(out=woutd[:], in_=wout_d[:])
        for m in range(MT):
            nc.sync.dma_start(out=ebt[:, m, :], in_=ebt_d[m * 128:(m + 1) * 128, :])
        nc.vector.memset(vsb[:], 1.0)

        # --- q/k projection (transposed orientation) ---
        with tc.tile_pool(name="ppsq", bufs=8, space="PSUM") as ppsq:
            for ch in range(NQ):
                for mb in range(2):       # 0 -> q block, 1 -> k block
                    ps = ppsq.tile([128, 512], f32, tag="qkps")
                    for kk in range(2):
                        nc.tensor.matmul(
                            ps[:],
                            lhsT=wqk[:, kk, mb * 128:(mb + 1) * 128],
                            rhs=xT[:, kk, ch * 512:(ch + 1) * 512],
                            start=(kk == 0), stop=(kk == 1),
                        )
                    nc.vector.tensor_copy(
                        out=qkT[:, mb, ch * 512:(ch + 1) * 512], in_=ps[:]
                    )

        # --- attention ---
        with tc.tile_pool(name="awp", bufs=2) as awp, \
             tc.tile_pool(name="aw2p", bufs=2) as aw2p, \
             tc.tile_pool(name="recp", bufs=2) as recp:
          with tc.tile_pool(name="sps", bufs=2, space="PSUM") as sps:

            def issue_scores(m, ncol0):
                scs = []
                for hp in range(2):
                    sc = sps.tile([128, 2, NQW], f32, tag="sc",
                                  name=f"sc{hp}")
                    for hi in range(2):
                        hl = hp * 2 + hi
                        nc.tensor.matmul(
                            sc[:, hi, :],
                            lhsT=qkT[32 * hl:32 * (hl + 1), 1,
                                     m * 128:(m + 1) * 128],
                            rhs=qkT[32 * hl:32 * (hl + 1), 0,
                                    ncol0:ncol0 + NQW],
                            start=True, stop=True,
                            tile_position=(32 * hl, 0),
                        )
                    scs.append(sc)
                return scs

            def issue_act_mul(m, ncol0, scs):
                aw = awp.tile([128, HL, NQW], f16, tag="aw")
                for hp in range(2):
                    nc.scalar.activation(
                        out=aw[:, 2 * hp:2 * hp + 2, :], in_=scs[hp][:],
                        func=mybir.ActivationFunctionType.Exp,
                        scale=SCALE,
                    )
                ebs = ebt[:, m, ncol0:ncol0 + NQW]
                eb_b = bass.AP(
                    tensor=ebs.tensor, offset=ebs.offset,
                    ap=[ebs.ap[0], [0, HL], ebs.ap[1]],
                )
                aw2 = aw2p.tile([128, HL, NQW], f16, tag="aw2")
                nc.vector.tensor_mul(aw2[:], aw[:], eb_b)
                return aw2

            def issue_av(m, aw2, oa):
                for hp in range(2):
                    for hi in range(2):
                        hl = hp * 2 + hi
                        po = 64 * (hl % 2)
                        nc.tensor.matmul(
                            oa[hl // 2][po:po + 64, :],
                            lhsT=vsb[:, m, hl, :],
                            rhs=aw2[:, hl, :],
                            start=(m == 0), stop=(m == MT - 1),
                        )

            # prologue: first scores + exp of quarter 0, THEN the
            # v-projection (tensor chews it while ACT runs)
            scs0 = issue_scores(0, 0)
            aw2s0 = issue_act_mul(0, 0, scs0)
            with tc.tile_pool(name="ppsv", bufs=4, space="PSUM") as ppsv:
                for nt in range(MT):
                    ps = ppsv.tile([128, 128], f32, tag="vps")
                    for kk in range(2):
                        nc.tensor.matmul(
                            ps[:],
                            lhsT=xT[:, kk, nt * 128:(nt + 1) * 128],
                            rhs=wv[:, kk, :],
                            start=(kk == 0), stop=(kk == 1),
                        )
                    nc.vector.tensor_copy(out=vsb[:, nt, :, 0:32], in_=ps[:])

            with tc.tile_pool(name="oap", bufs=2, space="PSUM") as oap:
                pending = []   # deferred normalize steps, previous quarter

                for q in range(NQ):
                    ncol0 = q * NQW
                    oa = [oap.tile([128, NQW], f32, tag=f"oa{i}",
                                   name=f"oa{i}_{q}")
                          for i in range(2)]

                    def make_norm_steps(qq, oa_q):
                        recs = [recp.tile([128, NQW], f32, tag=f"rec{i}",
                                          name=f"rec{i}_{qq}")
                                for i in range(2)]

                        def make_recip(i):
                            def steprec():
                                nc.vector.reciprocal_approx_fast(
                                    out=recs[i][:], in_=oa_q[i][:]
                                )
                            return steprec

                        def make_mul(hl):
                            def stepmul():
                                po = 64 * (hl % 2)
                                nc.vector.tensor_mul(
                                    aoutT[32 * hl:32 * hl + 32,
                                          qq * NQW:(qq + 1) * NQW],
                                    oa_q[hl // 2][po:po + 32, :],
                                    recs[hl // 2][po + 32:po + 64, :],
                                )
                            return stepmul

                        return ([make_recip(i) for i in range(2)]
                                + [make_mul(hl) for hl in range(HL)])

                    scs = scs0 if q == 0 else issue_scores(0, ncol0)
                    for m in range(MT):
                        if q == 0 and m == 0:
                            aw2 = aw2s0
                        else:
                            aw2 = issue_act_mul(m, ncol0, scs)
                        if m + 1 < MT:
                            scs = issue_scores(m + 1, ncol0)
                        issue_av(m, aw2, oa)
                        if pending and m >= 1:
                            pending.pop(0)()
                    pending = make_norm_steps(q, oa)

                # q3 normalize flush (inside the oa pool scope)
                for step in pending:
                    step()
                pending = []

          # output projection: all attention PSUM pools closed, banks free
          with tc.tile_pool(name="prj", bufs=4, space="PSUM") as prj, \
               tc.tile_pool(name="otp", bufs=4) as otp:
            for nt in range(MT):
                pp = prj.tile([128, OUP], f32)
                nc.tensor.matmul(
                    pp[:],
                    lhsT=aoutT[:, nt * 128:(nt + 1) * 128],
                    rhs=woutd[:],
                    start=True, stop=True,
                )
                ot = otp.tile([128, OUP], f32)
                if nt % 2 == 0:
                    nc.scalar.copy(out=ot[:], in_=pp[:])
                else:
                    nc.vector.tensor_copy(out=ot[:], in_=pp[:])
                nc.gpsimd.dma_start(
                    out=out_d[nt * 128:(nt + 1) * 128, :], in_=ot[:]
                )
    nc.compile()
    return nc


_NC_CACHE = {}


def kernel(x, w_qkv, bias_table, w_out, b_out, relative_pos):
    x = np.asarray(x, np.float32)
    w_qkv = np.asarray(w_qkv, np.float32)
    bias_table = np.asarray(bias_table, np.float32)
    w_out = np.asarray(w_out, np.float32)
    b_out = np.asarray(b_out, np.float32)
    relative_pos = np.asarray(relative_pos, np.int32)

    bias = bias_table[relative_pos, 0]                       # [n, m]
    expBT = np.exp(bias.T - C_SHIFT).astype(np.float16)      # [m, n]
    expBT = np.ascontiguousarray(expBT)

    if "nc" not in _NC_CACHE:
        _NC_CACHE["nc"] = _build_nc()
    nc = _NC_CACHE["nc"]

    in_maps = []
    for c in range(8):
        b, g = c // 2, c % 2
        w_qk = np.concatenate(
            [w_qkv[:, g * 128:(g + 1) * 128],
             w_qkv[:, 256 + g * 128:256 + (g + 1) * 128]], axis=1)
        in_maps.append({
            "xT": np.ascontiguousarray(x[b].T).reshape(2, 128, N).astype(np.float16),
            "w_qk": np.ascontiguousarray(w_qk).reshape(2, 128, 256).astype(np.float16),
            "w_v": np.ascontiguousarray(
                w_qkv[:, 512 + g * 128:512 + (g + 1) * 128]
            ).reshape(2, 128, 128).astype(np.float16),
            "w_out2": np.ascontiguousarray(
                w_out[g * 128:(g + 1) * 128, :]
            ).astype(np.float16),
            "expbt": expBT,
        })

    trace = bool(os.environ.get("KERNEL_TRACE"))
    res = run_bass_kernel_spmd(nc, in_maps, list(range(8)), trace=trace)
    _LAST["exec_time_ns"] = res.exec_time_ns
    _LAST["results"] = res

    parts = [np.asarray(res.results[c]["partial"], np.float32) for c in range(8)]
    out = np.stack([parts[2 * b] + parts[2 * b + 1] + b_out for b in range(B)])
    return out.astype(np.float32)


# revision 7
# speedup vs baseline: 1.1366x; 1.1366x over previous
"""Trainium2 Bass kernel for windowless 3D relative-position attention.

Full-input contract: kernel(**inputs) takes the unsharded numpy inputs and
returns the full [4, 2048, 256] output. Internally shards across 8 NeuronCores
as (batch b = core//2) x (head-group g = core%2, 4 heads each).

v2 design (ACT-exp is the pacemaker; PE assumed cold at 1.2 GHz due to HAM):
  - expBT resident in SBUF ([128, 16, 2048] fp16) loaded once.
  - scores per head-pair tile [128, 2, 512] fp32 (2 PSUM banks), one shared
    tag ring of 3 bufs (6 banks) -> the PE gets ~2 ACT-slots of lookahead
    and the 4 per-m score matmuls (row bands 0..3 via tile_position) fly
    concurrently while ACT drains earlier tiles.
  - ONE ACT exp per head-pair (FD=1024, PSUM->SBUF fp16), writing halves of
    a [128, 4, 512] aw tile; ONE DVE mul (FD=2048) applies exp(bias) to all
    4 heads per m-tile (eb broadcast via stride-0 AP).
  - AV lhsT is [128 keys, 64]: cols 0:32 = v, 32:64 = ones -> the softmax
    denominator comes out REPLICATED on 32 psum partitions; normalization is
    reciprocal_approx_fast per oa bank + one fp32 mul per head (no DRAM
    round-trips, no 1-partition copies).
  - oa: 2 heads per bank at PE column offsets 0/64 (2-way AV concurrency),
    2 banks per quarter, single-buffered. PSUM total = 6 (sc ring) + 2 = 8.
  - quarter-q normalize steps (2 recip + 4 mul) are flushed as one DVE
    burst at the start of quarter q+1, and AV matmuls trail the score
    stream by one m-slot, so the oa write-after-read wait never blocks
    score issue in the in-order PE queue (ACT never starves).
  - out-projection runs at the tail with copies alternating scalar/vector
    engines, output DMA on the gpsimd queue.
  - dummy exp activation issued at t~0 so the ~2.7us ACT table load hides
    under the input DMAs.

The bias add is factored through the exponential: exp(s+bias) =
exp(s)*exp(bias), with exp(bias.T - C_SHIFT) precomputed on host in fp16
(C_SHIFT keeps products in fp16 range; it cancels in the softmax ratio).

Hardware constraints learned (do not re-attempt blindly): matmul PSUM out
is limited to one 2KB bank (512 fp32 cols); engine APs must start on
partition 0/32/64/96 and fit the quadrant; DMA cannot touch PSUM; DMA
partition stride must be 1 (stride-0 broadcast only from DRAM); tile pools
release LIFO; walrus ldw-opt is incompatible with Bass IR; fp8 DoubleRow
gives no column-rate gain at K<128; GPSIMD DGE adds latency in
dependency-critical chains; HAM keeps PE at 1.2 GHz unless continuously
busy >=3.4us (ACT-gated cadence never re-warms it).
"""

import os
import sys
from contextlib import ExitStack

import numpy as np

sys.path.insert(0, "/opt/trn_rl_repo")

import concourse.bass as bass
import concourse.bacc as bacc
import concourse.tile as tile
from concourse import mybir
from concourse.bass_utils import run_bass_kernel_spmd

# Problem constants (hardcoded per contract)
B = 4
N = 2048
INP = 256
OUP = 256
HEADS = 8
DIM_HEAD = 32
SCALE = DIM_HEAD ** -0.5
HL = 4            # heads per core
MT = N // 128     # 16 m-tiles (keys)
NQ = 4            # 512-wide n (query) quarters
NQW = 512
C_SHIFT = 4.0

f32 = mybir.dt.float32
f16 = mybir.dt.float16

_LAST = {"exec_time_ns": None}


def _build_nc():
    nc = bacc.Bacc("TRN2", target_bir_lowering=False, debug=False)
    xT_d = nc.dram_tensor("xT", [2, 128, N], f16, kind="ExternalInput")
    wqk_d = nc.dram_tensor("w_qk", [2, 128, 256], f16, kind="ExternalInput")
    wv_d = nc.dram_tensor("w_v", [2, 128, 128], f16, kind="ExternalInput")
    wout_d = nc.dram_tensor("w_out2", [128, 256], f16, kind="ExternalInput")
    ebt_d = nc.dram_tensor("expbt", [N, N], f16, kind="ExternalInput")
    out_d = nc.dram_tensor("partial", [N, OUP], f32, kind="ExternalOutput")

    with ExitStack() as ctx:
        tc = ctx.enter_context(tile.TileContext(nc))
        consts = ctx.enter_context(tc.tile_pool(name="consts", bufs=1))

        ebt = consts.tile([128, MT, N], f16)          # [m%128, mtile, n]
        xT = consts.tile([128, 2, N], f16)
        wqk = consts.tile([128, 2, 256], f16)
        wv = consts.tile([128, 2, 128], f16)
        woutd = consts.tile([128, 256], f16)
        qkT = consts.tile([128, 2, N], f16)           # [:,0,:]=qT  [:,1,:]=kT
        vsb = consts.tile([128, MT, HL, 64], f16)     # [key, mtile, head, v|ones]
        aoutT = consts.tile([128, N], f16)            # [(h,d), n] normalized
        dummy = consts.tile([128, 8], f32)

        # dummy exp right away: the ~2.7us ACT table load hides under DMAs
        nc.vector.memset(dummy[:], 1.0)
        nc.scalar.activation(
            out=dummy[:, 0:4], in_=dummy[:, 4:8],
            func=mybir.ActivationFunctionType.Exp, scale=1.0,
        )

        # first columns of x + w_qk first (unblocks projection chunk 0),
        # then the rest; the big ebt stream follows behind
        for kk in range(2):
            nc.sync.dma_start(out=xT[:, kk, 0:N // 2], in_=xT_d[kk, :, 0:N // 2])
        for kk in range(2):
            nc.sync.dma_start(out=wqk[:, kk, :], in_=wqk_d[kk])
        for kk in range(2):
            nc.sync.dma_start(out=xT[:, kk, N // 2:N], in_=xT_d[kk, :, N // 2:N])
        for kk in range(2):
            nc.sync.dma_start(out=wv[:, kk, :], in_=wv_d[kk])
        nc.sync.dma_start(out=woutd[:], in_=wout_d[:])
        for m in range(MT):
            nc.sync.dma_start(out=ebt[:, m, :], in_=ebt_d[m * 128:(m + 1) * 128, :])
        nc.vector.memset(vsb[:], 1.0)

        # --- q/k projection (transposed orientation) ---
        with tc.tile_pool(name="ppsq", bufs=8, space="PSUM") as ppsq:
            for ch in range(NQ):
                for mb in range(2):       # 0 -> q block, 1 -> k block
                    ps = ppsq.tile([128, 512], f32, tag="qkps")
                    for kk in range(2):
                        nc.tensor.matmul(
                            ps[:],
                            lhsT=wqk[:, kk, mb * 128:(mb + 1) * 128],
                            rhs=xT[:, kk, ch * 512:(ch + 1) * 512],
                            start=(kk == 0), stop=(kk == 1),
                        )
                    nc.vector.tensor_copy(
                        out=qkT[:, mb, ch * 512:(ch + 1) * 512], in_=ps[:]
                    )

        # --- attention ---
        with tc.tile_pool(name="awp", bufs=3) as awp, \
             tc.tile_pool(name="aw2p", bufs=3) as aw2p, \
             tc.tile_pool(name="recp", bufs=2) as recp:
          with tc.tile_pool(name="sps", bufs=3, space="PSUM") as sps:

            def issue_scores(m, ncol0):
                scs = []
                for hp in range(2):
                    sc = sps.tile([128, 2, NQW], f32, tag="sc",
                                  name=f"sc{hp}")
                    for hi in range(2):
                        hl = hp * 2 + hi
                        nc.tensor.matmul(
                            sc[:, hi, :],
                            lhsT=qkT[32 * hl:32 * (hl + 1), 1,
                                     m * 128:(m + 1) * 128],
                            rhs=qkT[32 * hl:32 * (hl + 1), 0,
                                    ncol0:ncol0 + NQW],
                            start=True, stop=True,
                            tile_position=(32 * hl, 0),
                        )
                    scs.append(sc)
                return scs

            def issue_act_mul(m, ncol0, scs):
                aw = awp.tile([128, HL, NQW], f16, tag="aw")
                for hp in range(2):
                    nc.scalar.activation(
                        out=aw[:, 2 * hp:2 * hp + 2, :], in_=scs[hp][:],
                        func=mybir.ActivationFunctionType.Exp,
                        scale=SCALE,
                    )
                ebs = ebt[:, m, ncol0:ncol0 + NQW]
                eb_b = bass.AP(
                    tensor=ebs.tensor, offset=ebs.offset,
                    ap=[ebs.ap[0], [0, HL], ebs.ap[1]],
                )
                aw2 = aw2p.tile([128, HL, NQW], f16, tag="aw2")
                nc.vector.tensor_mul(aw2[:], aw[:], eb_b)
                return aw2

            def issue_av(m, aw2, oa):
                for hp in range(2):
                    for hi in range(2):
                        hl = hp * 2 + hi
                        po = 64 * (hl % 2)
                        nc.tensor.matmul(
                            oa[hl // 2][po:po + 64, :],
                            lhsT=vsb[:, m, hl, :],
                            rhs=aw2[:, hl, :],
                            start=(m == 0), stop=(m == MT - 1),
                        )

            # prologue: first scores + exp of quarter 0, THEN the
            # v-projection (tensor chews it while ACT runs)
            scs0 = issue_scores(0, 0)
            aw2s0 = issue_act_mul(0, 0, scs0)
            with tc.tile_pool(name="ppsv", bufs=2, space="PSUM") as ppsv:
                for nt in range(MT):
                    ps = ppsv.tile([128, 128], f32, tag="vps")
                    for kk in range(2):
                        nc.tensor.matmul(
                            ps[:],
                            lhsT=xT[:, kk, nt * 128:(nt + 1) * 128],
                            rhs=wv[:, kk, :],
                            start=(kk == 0), stop=(kk == 1),
                        )
                    nc.vector.tensor_copy(out=vsb[:, nt, :, 0:32], in_=ps[:])

            with tc.tile_pool(name="oap", bufs=1, space="PSUM") as oap:
                pending = []   # deferred normalize steps, previous quarter

                def issue_norm(qq, oa_q):
                    recs = [recp.tile([128, NQW], f32, tag=f"rec{i}",
                                      name=f"rec{i}_{qq}")
                            for i in range(2)]
                    for i in range(2):
                        nc.vector.reciprocal_approx_fast(
                            out=recs[i][:], in_=oa_q[i][:]
                        )
                    for hl in range(HL):
                        po = 64 * (hl % 2)
                        nc.vector.tensor_mul(
                            aoutT[32 * hl:32 * hl + 32,
                                  qq * NQW:(qq + 1) * NQW],
                            oa_q[hl // 2][po:po + 32, :],
                            recs[hl // 2][po + 32:po + 64, :],
                        )

                for q in range(NQ):
                    ncol0 = q * NQW
                    oa = [oap.tile([128, NQW], f32, tag=f"oa{i}",
                                   name=f"oa{i}_{q}")
                          for i in range(2)]

                    scs = scs0 if q == 0 else issue_scores(0, ncol0)
                    av_back = []   # AVs trail scores by one m-slot
                    for m in range(MT):
                        if q == 0 and m == 0:
                            aw2 = aw2s0
                        else:
                            aw2 = issue_act_mul(m, ncol0, scs)
                        if m + 1 < MT:
                            scs = issue_scores(m + 1, ncol0)
                        if m == 0 and pending:
                            # flush previous quarter's normalize as one DVE
                            # burst before any AV of this quarter is issued
                            issue_norm(*pending)
                            pending = []
                        av_back.append((m, aw2))
                        if len(av_back) > 1:
                            issue_av(*av_back.pop(0), oa)
                    for it in av_back:
                        issue_av(*it, oa)
                    pending = (q, oa)

                # q3 normalize flush (inside the oa pool scope)
                if pending:
                    issue_norm(*pending)
                    pending = None

          # output projection: all attention PSUM pools closed, banks free
          with tc.tile_pool(name="prj", bufs=4, space="PSUM") as prj, \
               tc.tile_pool(name="otp", bufs=4) as otp:
            for nt in range(MT):
                pp = prj.tile([128, OUP], f32)
                nc.tensor.matmul(
                    pp[:],
                    lhsT=aoutT[:, nt * 128:(nt + 1) * 128],
                    rhs=woutd[:],
                    start=True, stop=True,
                )
                ot = otp.tile([128, OUP], f32)
                if nt % 2 == 0:
                    nc.scalar.copy(out=ot[:], in_=pp[:])
                else:
                    nc.vector.tensor_copy(out=ot[:], in_=pp[:])
                nc.gpsimd.dma_start(
                    out=out_d[nt * 128:(nt + 1) * 128, :], in_=ot[:]
                )
    nc.compile()
    return nc


_NC_CACHE = {}


def kernel(x, w_qkv, bias_table, w_out, b_out, relative_pos):
    x = np.asarray(x, np.float32)
    w_qkv = np.asarray(w_qkv, np.float32)
    bias_table = np.asarray(bias_table, np.float32)
    w_out = np.asarray(w_out, np.float32)
    b_out = np.asarray(b_out, np.float32)
    relative_pos = np.asarray(relative_pos, np.int32)

    bias = bias_table[relative_pos, 0]                       # [n, m]
    expBT = np.exp(bias.T - C_SHIFT).astype(np.float16)      # [m, n]
    expBT = np.ascontiguousarray(expBT)

    if "nc" not in _NC_CACHE:
        _NC_CACHE["nc"] = _build_nc()
    nc = _NC_CACHE["nc"]

    in_maps = []
    for c in range(8):
        b, g = c // 2, c % 2
        w_qk = np.concatenate(
            [w_qkv[:, g * 128:(g + 1) * 128],
             w_qkv[:, 256 + g * 128:256 + (g + 1) * 128]], axis=1)
        in_maps.append({
            "xT": np.ascontiguousarray(x[b].T).reshape(2, 128, N).astype(np.float16),
            "w_qk": np.ascontiguousarray(w_qk).reshape(2, 128, 256).astype(np.float16),
            "w_v": np.ascontiguousarray(
                w_qkv[:, 512 + g * 128:512 + (g + 1) * 128]
            ).reshape(2, 128, 128).astype(np.float16),
            "w_out2": np.ascontiguousarray(
                w_out[g * 128:(g + 1) * 128, :]
            ).astype(np.float16),
            "expbt": expBT,
        })

    trace = bool(os.environ.get("KERNEL_TRACE"))
    res = run_bass_kernel_spmd(nc, in_maps, list(range(8)), trace=trace)
    _LAST["exec_time_ns"] = res.exec_time_ns
    _LAST["results"] = res

    parts = [np.asarray(res.results[c]["partial"], np.float32) for c in range(8)]
    out = np.stack([parts[2 * b] + parts[2 * b + 1] + b_out for b in range(B)])
    return out.astype(np.float32)


# revision 12
# speedup vs baseline: 1.1576x; 1.0185x over previous
"""Trainium2 Bass kernel for windowless 3D relative-position attention.

Full-input contract: kernel(**inputs) takes the unsharded numpy inputs and
returns the full [4, 2048, 256] output. Internally shards across 8 NeuronCores
as (batch b = core//2) x (head-group g = core%2, 4 heads each).

v2 design (ACT-exp is the pacemaker; PE assumed cold at 1.2 GHz due to HAM):
  - expBT resident in SBUF ([128, 16, 2048] fp16) loaded once.
  - scores per head-pair tile [128, 2, 512] fp32 (2 PSUM banks), one shared
    tag ring of 3 bufs (6 banks) -> the PE gets ~2 ACT-slots of lookahead
    and the 4 per-m score matmuls (row bands 0..3 via tile_position) fly
    concurrently while ACT drains earlier tiles.
  - ONE ACT exp per head-pair (FD=1024, PSUM->SBUF fp16), writing halves of
    a [128, 4, 512] aw tile; ONE DVE mul (FD=2048) applies exp(bias) to all
    4 heads per m-tile (eb broadcast via stride-0 AP).
  - AV lhsT is [128 keys, 64]: cols 0:32 = v, 32:64 = ones -> the softmax
    denominator comes out REPLICATED on 32 psum partitions; normalization is
    reciprocal_approx_fast per oa bank + one fp32 mul per head (no DRAM
    round-trips, no 1-partition copies).
  - oa: 2 heads per bank at PE column offsets 0/64 (2-way AV concurrency),
    2 banks per quarter, single-buffered. PSUM total = 6 (sc ring) + 2 = 8.
  - quarter-q normalize steps (2 recip + 4 mul) are flushed as one DVE
    burst at the start of quarter q+1, and AV matmuls trail the score
    stream by one m-slot, so the oa write-after-read wait never blocks
    score issue in the in-order PE queue (ACT never starves).
  - out-projection runs at the tail with copies alternating scalar/vector
    engines, output DMA on the gpsimd queue.
  - dummy exp activation issued at t~0 so the ~2.7us ACT table load hides
    under the input DMAs.

The bias add is factored through the exponential: exp(s+bias) =
exp(s)*exp(bias), with exp(bias.T - C_SHIFT) precomputed on host in fp16
(C_SHIFT keeps products in fp16 range; it cancels in the softmax ratio).

Hardware constraints learned (do not re-attempt blindly): matmul PSUM out
is limited to one 2KB bank (512 fp32 cols); engine APs must start on
partition 0/32/64/96 and fit the quadrant; DMA cannot touch PSUM; DMA
partition stride must be 1 (stride-0 broadcast only from DRAM); tile pools
release LIFO; walrus ldw-opt is incompatible with Bass IR; fp8 DoubleRow
gives no column-rate gain at K<128; GPSIMD DGE adds latency in
dependency-critical chains; HAM keeps PE at 1.2 GHz unless continuously
busy >=3.4us (ACT-gated cadence never re-warms it).
"""

import os
import sys
from contextlib import ExitStack

import numpy as np

sys.path.insert(0, "/opt/trn_rl_repo")

import concourse.bass as bass
import concourse.bacc as bacc
import concourse.tile as tile
from concourse import mybir
from concourse.bass_utils import run_bass_kernel_spmd

# Problem constants (hardcoded per contract)
B = 4
N = 2048
INP = 256
OUP = 256
HEADS = 8
DIM_HEAD = 32
SCALE = DIM_HEAD ** -0.5
HL = 4            # heads per core
MT = N // 128     # 16 m-tiles (keys)
NQ = 4            # 512-wide n (query) quarters
NQW = 512
C_SHIFT = 4.0

f32 = mybir.dt.float32
f16 = mybir.dt.float16

_LAST = {"exec_time_ns": None}


def _build_nc():
    nc = bacc.Bacc("TRN2", target_bir_lowering=False, debug=False)
    xT_d = nc.dram_tensor("xT", [2, 128, N], f16, kind="ExternalInput")
    wqk_d = nc.dram_tensor("w_qk", [2, 128, 256], f16, kind="ExternalInput")
    wv_d = nc.dram_tensor("w_v", [2, 128, 128], f16, kind="ExternalInput")
    wout_d = nc.dram_tensor("w_out2", [128, 256], f16, kind="ExternalInput")
    ebt_d = nc.dram_tensor("expbt", [N, N], f16, kind="ExternalInput")
    out_d = nc.dram_tensor("partial", [N, OUP], f32, kind="ExternalOutput")

    with ExitStack() as ctx:
        tc = ctx.enter_context(tile.TileContext(nc))
        consts = ctx.enter_context(tc.tile_pool(name="consts", bufs=1))

        ebt = consts.tile([128, MT, N], f16)          # [m%128, mtile, n]
        xT = consts.tile([128, 2, N], f16)
        wqk = consts.tile([128, 2, 256], f16)
        wv = consts.tile([128, 2, 128], f16)
        woutd = consts.tile([128, 256], f16)
        qkT = consts.tile([128, 2, N], f16)           # [:,0,:]=qT  [:,1,:]=kT
        vsb = consts.tile([128, MT, HL, 64], f16)     # [key, mtile, head, v|ones]
        aoutT = consts.tile([128, N], f16)            # [(h,d), n] normalized
        dummy = consts.tile([128, 8], f32)

        # dummy exp right away: the ~2.7us ACT table load hides under DMAs
        nc.vector.memset(dummy[:], 1.0)
        nc.scalar.activation(
            out=dummy[:, 0:4], in_=dummy[:, 4:8],
            func=mybir.ActivationFunctionType.Exp, scale=1.0,
        )

        # DMA order tuned so the first-scores critical path (x first halves +
        # w_qk -> qk-proj ch0 -> scores(t0) -> exp) unblocks ~4us in, and
        # ebt tiles 0-1 land before their DVE muls need them
        for kk in range(2):
            nc.sync.dma_start(out=xT[:, kk, 0:N // 2], in_=xT_d[kk, :, 0:N // 2])
        for kk in range(2):
            nc.sync.dma_start(out=wqk[:, kk, :], in_=wqk_d[kk])
        for m in range(2):
            nc.sync.dma_start(out=ebt[:, m, :], in_=ebt_d[m * 128:(m + 1) * 128, :])
        for kk in range(2):
            nc.sync.dma_start(out=xT[:, kk, N // 2:N], in_=xT_d[kk, :, N // 2:N])
        for kk in range(2):
            nc.sync.dma_start(out=wv[:, kk, :], in_=wv_d[kk])
        nc.sync.dma_start(out=woutd[:], in_=wout_d[:])
        for m in range(2, MT):
            nc.sync.dma_start(out=ebt[:, m, :], in_=ebt_d[m * 128:(m + 1) * 128, :])
        nc.vector.memset(vsb[:], 1.0)

        # --- attention (projections interleaved into the early q0 stream) ---
        with tc.tile_pool(name="awp", bufs=3) as awp, \
             tc.tile_pool(name="aw2p", bufs=6) as aw2p, \
             tc.tile_pool(name="recp", bufs=2) as recp:
          with tc.tile_pool(name="sps", bufs=3, space="PSUM") as sps:

            def issue_scores(m, ncol0):
                scs = []
                for hp in range(2):
                    sc = sps.tile([128, 2, NQW], f32, tag="sc",
                                  name=f"sc{hp}")
                    for hi in range(2):
                        hl = hp * 2 + hi
                        nc.tensor.matmul(
                            sc[:, hi, :],
                            lhsT=qkT[32 * hl:32 * (hl + 1), 1,
                                     m * 128:(m + 1) * 128],
                            rhs=qkT[32 * hl:32 * (hl + 1), 0,
                                    ncol0:ncol0 + NQW],
                            start=True, stop=True,
                            tile_position=(32 * hl, 0),
                        )
                    scs.append(sc)
                return scs

            def issue_act_mul(m, ncol0, scs):
                aw = awp.tile([128, HL, NQW], f16, tag="aw")
                for hp in range(2):
                    nc.scalar.activation(
                        out=aw[:, 2 * hp:2 * hp + 2, :], in_=scs[hp][:],
                        func=mybir.ActivationFunctionType.Exp,
                        scale=SCALE,
                    )
                ebs = ebt[:, m, ncol0:ncol0 + NQW]
                eb_b = bass.AP(
                    tensor=ebs.tensor, offset=ebs.offset,
                    ap=[ebs.ap[0], [0, HL], ebs.ap[1]],
                )
                aw2 = aw2p.tile([128, HL, NQW], f16, tag="aw2")
                nc.vector.tensor_mul(aw2[:], aw[:], eb_b)
                return aw2

            def issue_av(m, aw2, oa):
                for hp in range(2):
                    for hi in range(2):
                        hl = hp * 2 + hi
                        po = 64 * (hl % 2)
                        nc.tensor.matmul(
                            oa[hl // 2][po:po + 64, :],
                            lhsT=vsb[:, m, hl, :],
                            rhs=aw2[:, hl, :],
                            start=(m == 0), stop=(m == MT - 1),
                        )

            # prologue: qk-projection chunks, v-projection tiles and the
            # first six m-tiles' scores+exp of quarter 0 are interleaved so
            # ACT starts ~5us in and never gaps while the PE does the
            # projections. AVs for these tiles are deferred into the main
            # loop (av_back) and drained 2/slot.
            pro_scs = []      # scores tiles t0..t5
            pro_aw2 = []      # aw2 tiles t0..t4
            with tc.tile_pool(name="ppsq", bufs=2, space="PSUM") as ppsq:
                def qk_chunk(ch):
                    for mb in range(2):   # 0 -> q block, 1 -> k block
                        ps = ppsq.tile([128, 512], f32, tag="qkps",
                                       name=f"qkps{ch}_{mb}")
                        for kk in range(2):
                            nc.tensor.matmul(
                                ps[:],
                                lhsT=wqk[:, kk, mb * 128:(mb + 1) * 128],
                                rhs=xT[:, kk, ch * 512:(ch + 1) * 512],
                                start=(kk == 0), stop=(kk == 1),
                            )
                        nc.vector.tensor_copy(
                            out=qkT[:, mb, ch * 512:(ch + 1) * 512], in_=ps[:]
                        )

                def pro_tile(t):
                    if t >= 1:   # t-1's exp/mul before t's scores (sc ring
                        pro_aw2.append(issue_act_mul(t - 1, 0, pro_scs[t - 1]))
                    pro_scs.append(issue_scores(t, 0))

                qk_chunk(0)
                pro_tile(0)
                pro_tile(1)
                qk_chunk(1)
                pro_tile(2)
                qk_chunk(2)
                qk_chunk(3)
                pro_tile(3)

            with tc.tile_pool(name="ppsv", bufs=2, space="PSUM") as ppsv:
                def v_tile(nt):
                    ps = ppsv.tile([128, 128], f32, tag="vps",
                                   name=f"vps{nt}")
                    for kk in range(2):
                        nc.tensor.matmul(
                            ps[:],
                            lhsT=xT[:, kk, nt * 128:(nt + 1) * 128],
                            rhs=wv[:, kk, :],
                            start=(kk == 0), stop=(kk == 1),
                        )
                    nc.vector.tensor_copy(out=vsb[:, nt, :, 0:32], in_=ps[:])

                for nt in range(8):
                    v_tile(nt)
                pro_tile(4)
                for nt in range(8, MT):
                    v_tile(nt)
                pro_tile(5)

            with tc.tile_pool(name="oap", bufs=1, space="PSUM") as oap:
                pending = []   # deferred normalize steps, previous quarter

                def issue_norm(qq, oa_q):
                    recs = [recp.tile([128, NQW], f32, tag=f"rec{i}",
                                      name=f"rec{i}_{qq}")
                            for i in range(2)]
                    for i in range(2):
                        nc.vector.reciprocal_approx_fast(
                            out=recs[i][:], in_=oa_q[i][:]
                        )
                    for hl in range(HL):
                        po = 64 * (hl % 2)
                        nc.vector.tensor_mul(
                            aoutT[32 * hl:32 * hl + 32,
                                  qq * NQW:(qq + 1) * NQW],
                            oa_q[hl // 2][po:po + 32, :],
                            recs[hl // 2][po + 32:po + 64, :],
                        )

                next_scs = None
                for q in range(NQ):
                    ncol0 = q * NQW
                    oa = [oap.tile([128, NQW], f32, tag=f"oa{i}",
                                   name=f"oa{i}_{q}")
                          for i in range(2)]

                    if q == 0:
                        # prologue pre-issued scores t0..t5, exp/mul t0..t4
                        scs = pro_scs[5]
                        av_back = [(t, pro_aw2[t]) for t in range(5)]
                        m0 = 5
                    else:
                        scs = next_scs
                        av_back = []
                        m0 = 0
                    for m in range(m0, MT):
                        aw2 = issue_act_mul(m, ncol0, scs)
                        if m + 1 < MT:
                            scs = issue_scores(m + 1, ncol0)
                        elif q + 1 < NQ:
                            # next quarter's first scores go ahead of the
                            # AV backlog flush so ACT never gaps
                            next_scs = issue_scores(0, ncol0 + NQW)
                        if m == m0 and pending:
                            # flush previous quarter's normalize as one DVE
                            # burst before any AV of this quarter is issued
                            issue_norm(*pending)
                            pending = []
                        av_back.append((m, aw2))
                        if len(av_back) > 1:
                            issue_av(*av_back.pop(0), oa)
                        if len(av_back) > 1:
                            issue_av(*av_back.pop(0), oa)
                    for it in av_back:
                        issue_av(*it, oa)
                    pending = (q, oa)

                # q3 normalize flush (inside the oa pool scope)
                if pending:
                    issue_norm(*pending)
                    pending = None

          # output projection: all attention PSUM pools closed, banks free
          with tc.tile_pool(name="prj", bufs=4, space="PSUM") as prj, \
               tc.tile_pool(name="otp", bufs=4) as otp:
            for nt in range(MT):
                pp = prj.tile([128, OUP], f32)
                nc.tensor.matmul(
                    pp[:],
                    lhsT=aoutT[:, nt * 128:(nt + 1) * 128],
                    rhs=woutd[:],
                    start=True, stop=True,
                )
                ot = otp.tile([128, OUP], f32)
                if nt % 2 == 0:
                    nc.scalar.copy(out=ot[:], in_=pp[:])
                else:
                    nc.vector.tensor_copy(out=ot[:], in_=pp[:])
                nc.gpsimd.dma_start(
                    out=out_d[nt * 128:(nt + 1) * 128, :], in_=ot[:]
                )
    nc.compile()
    return nc


_NC_CACHE = {}


def kernel(x, w_qkv, bias_table, w_out, b_out, relative_pos):
    x = np.asarray(x, np.float32)
    w_qkv = np.asarray(w_qkv, np.float32)
    bias_table = np.asarray(bias_table, np.float32)
    w_out = np.asarray(w_out, np.float32)
    b_out = np.asarray(b_out, np.float32)
    relative_pos = np.asarray(relative_pos, np.int32)

    bias = bias_table[relative_pos, 0]                       # [n, m]
    expBT = np.exp(bias.T - C_SHIFT).astype(np.float16)      # [m, n]
    expBT = np.ascontiguousarray(expBT)

    if "nc" not in _NC_CACHE:
        _NC_CACHE["nc"] = _build_nc()
    nc = _NC_CACHE["nc"]

    in_maps = []
    for c in range(8):
        b, g = c // 2, c % 2
        w_qk = np.concatenate(
            [w_qkv[:, g * 128:(g + 1) * 128],
             w_qkv[:, 256 + g * 128:256 + (g + 1) * 128]], axis=1)
        in_maps.append({
            "xT": np.ascontiguousarray(x[b].T).reshape(2, 128, N).astype(np.float16),
            "w_qk": np.ascontiguousarray(w_qk).reshape(2, 128, 256).astype(np.float16),
            "w_v": np.ascontiguousarray(
                w_qkv[:, 512 + g * 128:512 + (g + 1) * 128]
            ).reshape(2, 128, 128).astype(np.float16),
            "w_out2": np.ascontiguousarray(
                w_out[g * 128:(g + 1) * 128, :]
            ).astype(np.float16),
            "expbt": expBT,
        })

    trace = bool(os.environ.get("KERNEL_TRACE"))
    res = run_bass_kernel_spmd(nc, in_maps, list(range(8)), trace=trace)
    _LAST["exec_time_ns"] = res.exec_time_ns
    _LAST["results"] = res

    parts = [np.asarray(res.results[c]["partial"], np.float32) for c in range(8)]
    out = np.stack([parts[2 * b] + parts[2 * b + 1] + b_out for b in range(B)])
    return out.astype(np.float32)
